# revision 14
# baseline (speedup 1.0000x reference)
"""AQT int8-quantized matmul (dynamic symmetric quantization) on 8 TRN2 cores.

Full problem: lhs [8192, 4096] f32 @ rhs [4096, 4096] f32 with per-row lhs
scales and per-column rhs scales (abs-max / 127.5), int8 round+clip, int32
matmul, dequantize by the outer product of scales.

Sharding: 2x4 grid over (M, N). Each core gets lhs rows M/2 and rhs cols N/4,
computes its [4096, 1024] output block; host assembles the 8 blocks. Both
quantization axes (lhs rows = per-row over full K, rhs cols = per-column over
full K) keep their full contraction dim on every core, so per-core results
match the unsharded reference exactly. No collectives needed.

Per-core kernel (build_aqt): quantized values are exact integers in
[-127, 127] stored as bf16; TensorE matmul with fp32 PSUM accumulation
reproduces the int32 matmul to ~1e-5. round() is exact round-half-even via
the +1.5*2^23 magic-constant trick (fp32 add/sub). Instead of a post-round
clip, the quant divisor is shrunk by (1-2^-20), which provably keeps rounded
values inside [-127, 127] and matches the reference's round-then-clip on the
abs-max elements; dequant uses the same shrunk divisor (5e-7 systematic
error). rhs per-column absmax runs as an elementwise max over k-tiles (ACT
Abs + DVE max) followed by one GpSimd partition_all_reduce(absmax), which
also broadcasts the column maxima to all partitions. lhs is quantized in
natural [M, K] layout (per-partition row scales on ScalarE), then moved to
[K, M] via DMA-xbar transpose in bf16. Engine split: DVE = reductions,
tensor_tensor, round tensor_scalar; ScalarE = Abs / scale+round-bias copies /
PSUM eviction with per-row dequant scale; GpSimd = partition_all_reduce only
(its elementwise ops are ~17x slower than DVE and contend for DVE's SBUF
port). Matmuls run kt-outer/nb-inner so each stationary (lhsT) tile feeds
both n-blocks.
"""
import sys

if "/opt/trn_rl_repo" not in sys.path:
    sys.path.insert(0, "/opt/trn_rl_repo")

from contextlib import ExitStack

import numpy as np

from concourse import bacc, bass_isa, mybir, tile
from concourse.bass_utils import run_bass_kernel_spmd

f32 = mybir.dt.float32
bf16 = mybir.dt.bfloat16
Alu = mybir.AluOpType
Act = mybir.ActivationFunctionType

P = 128
C_MAGIC = 1.5 * 2 ** 23
QDIV = 127.5 * (1.0 - 2.0 ** -20)
INV_QDIV = 1.0 / QDIV
TINY = 1e-30

M, K, N = 8192, 4096, 4096
MG, NG = 2, 4                      # shard grid rows (M) x cols (N)
M_loc, N_loc = M // MG, N // NG    # 4096, 1024 per core
N_CORES = MG * NG


def build_aqt(nc, M_loc, K, N_loc, W=512):
    KT, MT, NB = K // P, M_loc // P, N_loc // W

    lhs = nc.declare_dram_parameter("lhs", [M_loc, K], f32, isOutput=False)
    rhs = nc.declare_dram_parameter("rhs", [K, N_loc], f32, isOutput=False)
    out = nc.declare_dram_parameter("out", [M_loc, N_loc], f32, isOutput=True)

    with tile.TileContext(nc) as tc, ExitStack() as ctx:
        pool = lambda name, bufs: ctx.enter_context(tc.tile_pool(name=name, bufs=bufs))
        qr_pool = pool("qr", NB * KT)      # quantized rhs, resident
        sbc_pool = pool("sbc", NB)         # rhs dequant scales, resident
        rstage = pool("rstage", 3)         # rhs raw pass A
        rstage2 = pool("rstage2", 4)       # rhs raw pass B
        rmul = pool("rmul", 3)             # |rhs| / rhs * r_bc
        racc = pool("racc", 2)             # absmax accumulator ping-pong
        rbc = pool("rbc", 2)               # amax_bc / r_bc
        lraw = pool("lraw", 2)             # lhs raw [P, K] f32
        lt1 = pool("lt1", 1)               # lhs scaled+C [P, K] f32
        lqb = pool("lqb", 2)               # lhs quantized [P, K] bf16
        lqt = pool("lqt", 3)               # lhs quantized transposed [P, KT, P]
        lsc = pool("lsc", 1)               # s_l columns, resident
        lam = pool("lam", 4)               # [P, 1] scratch
        opool = pool("o1", 2)
        opool2 = pool("o2", 2)
        psum = ctx.enter_context(tc.tile_pool(name="psum", bufs=6, space="PSUM"))

        s_l_all = lsc.tile([P, MT], f32)

        # ---- rhs: absmax -> scales -> quantize (all-resident q_r) ----
        qr_tiles = {}
        sbc_tiles = {}
        for nb in range(NB):
            cs = slice(nb * W, (nb + 1) * W)
            acc = None
            for kt in range(KT):
                t = rstage.tile([P, W], f32, name="rstage")
                nc.sync.dma_start(t[:], rhs[kt * P:(kt + 1) * P, cs])
                ta = rmul.tile([P, W], f32, name="rabs")
                nc.scalar.activation(ta[:], t[:], Act.Abs)
                nacc = racc.tile([P, W], f32, name="racc")
                nc.vector.tensor_tensor(nacc[:], (acc or ta)[:], ta[:], op=Alu.max)
                acc = nacc
            amax = rbc.tile([P, W], f32, name="amax")
            nc.gpsimd.partition_all_reduce(amax[:], acc[:], channels=P,
                                           reduce_op=bass_isa.ReduceOp.absmax)
            s_bc = sbc_pool.tile([P, W], f32, name="sbc")
            nc.vector.tensor_scalar(s_bc[:], amax[:], TINY, INV_QDIV,
                                    op0=Alu.max, op1=Alu.mult)
            sbc_tiles[nb] = s_bc
            r_bc = rbc.tile([P, W], f32, name="rbc")
            nc.vector.reciprocal(r_bc[:], s_bc[:])
            for kt in range(KT):
                t2 = rstage2.tile([P, W], f32, name="rstage2")
                nc.sync.dma_start(t2[:], rhs[kt * P:(kt + 1) * P, cs])
                u = rmul.tile([P, W], f32, name="rmul")
                nc.vector.tensor_tensor(u[:], t2[:], r_bc[:], op=Alu.mult)
                q = qr_pool.tile([P, W], bf16, name="qr")
                nc.vector.tensor_scalar(q[:], u[:], C_MAGIC, C_MAGIC,
                                        op0=Alu.add, op1=Alu.subtract)
                qr_tiles[(nb, kt)] = q

        # ---- lhs quantize + transpose + matmul + dequant, per m-tile ----
        for mi in range(MT):
            rs = slice(mi * P, (mi + 1) * P)
            raw = lraw.tile([P, K], f32, name="lraw")
            nc.sync.dma_start(raw[:], lhs[rs, :])
            am = lam.tile([P, 1], f32, name="lam")
            nc.vector.tensor_reduce(am[:], raw[:], axis=mybir.AxisListType.X,
                                    op=Alu.max, apply_absolute_value=True)
            s_col = s_l_all[:, mi:mi + 1]
            nc.vector.tensor_scalar(s_col, am[:], TINY, INV_QDIV,
                                    op0=Alu.max, op1=Alu.mult)
            r_l = lam.tile([P, 1], f32, name="rl")
            nc.vector.reciprocal(r_l[:], s_col)
            t1 = lt1.tile([P, K], f32, name="lt1")
            nc.scalar.activation(t1[:], raw[:], Act.Copy, bias=C_MAGIC, scale=r_l[:])
            qb = lqb.tile([P, K], bf16, name="lqb")
            nc.scalar.activation(qb[:], t1[:], Act.Copy, bias=-C_MAGIC)
            qt = lqt.tile([P, KT, P], bf16, name="lqt")
            nc.sync.dma_start_transpose(qt[:], qb[:])

            # kt outer / nb inner: each stationary weight tile feeds NB matmuls
            pss = [psum.tile([P, W], f32, name="ps") for _ in range(NB)]
            for kt in range(KT):
                for nb in range(NB):
                    nc.tensor.matmul(pss[nb][:], qt[:, kt, :], qr_tiles[(nb, kt)][:],
                                     start=(kt == 0), stop=(kt == KT - 1))
            for nb in range(NB):
                ps = pss[nb]
                o1 = opool.tile([P, W], f32, name="o1")
                nc.scalar.activation(o1[:], ps[:], Act.Copy, bias=0.0,
                                     scale=s_l_all[:, mi:mi + 1])
                o2 = opool2.tile([P, W], f32, name="o2")
                nc.vector.tensor_tensor(o2[:], o1[:], sbc_tiles[nb][:], op=Alu.mult)
                nc.sync.dma_start(out[rs, nb * W:(nb + 1) * W], o2[:])
    return nc


_COMPILED_NC = None


def _get_compiled():
    global _COMPILED_NC
    if _COMPILED_NC is None:
        nc = bacc.Bacc("TRN2", target_bir_lowering=False, debug=False,
                       num_devices=N_CORES)
        build_aqt(nc, M_loc, K, N_loc)
        nc.compile()
        _COMPILED_NC = nc
    return _COMPILED_NC


def _shard(lhs, rhs):
    in_maps = []
    for i in range(N_CORES):
        mg, ng = divmod(i, NG)
        in_maps.append({
            "lhs": np.ascontiguousarray(lhs[mg * M_loc:(mg + 1) * M_loc, :]),
            "rhs": np.ascontiguousarray(rhs[:, ng * N_loc:(ng + 1) * N_loc]),
        })
    return in_maps


def kernel(lhs, rhs, _trace=False, _trace_kwargs=None):
    lhs = np.asarray(lhs, np.float32)
    rhs = np.asarray(rhs, np.float32)
    nc = _get_compiled()
    res = run_bass_kernel_spmd(nc, _shard(lhs, rhs), core_ids=list(range(N_CORES)),
                               trace=_trace, **(_trace_kwargs or {}))
    out = np.empty((M, N), np.float32)
    for i in range(N_CORES):
        mg, ng = divmod(i, NG)
        out[mg * M_loc:(mg + 1) * M_loc, ng * N_loc:(ng + 1) * N_loc] = \
            res.results[i]["out"]
    kernel.last_result = res
    return out
